# revision 1
# baseline (speedup 1.0000x reference)
"""Llama GQA attention (B=4,S=1024,H=4096,NH=32,NKV=8,D=128) on 8 TRN2 cores.

Strategy: tensor-parallel over heads (4 q heads + 1 kv head per core), host
all-reduce of o_proj partials.  v2: single fused pipeline tuned for PE
occupancy against the TimelineSim cost model.

Per 512-token tile n (batch b = n//2, half = n%2):
  1. qkv^T matmuls (bf16, m-outer k-inner, 2 rotating PSUM banks)
  2. RoPE epilogues per m as banks complete (DVE for q heads, Act for k;
     4-op rotate-half form using stacked [cos;cos] / [-sin;sin] tables)
  3. causal attention per head with column-trimmed score/PV matmuls,
     exp on Act (bf16 probs), diagonal-block mask on DVE, softmax
     denominator accumulated on the (otherwise idle) Pool engine
  4. o_proj matmuls of tile n-1 interleaved between attention matmuls to
     hide exp/mask latency; evictions rotate over Act/DVE/Pool; bf16
     output staged and DMA'd in 4-m-tile batches.
"""

import numpy as np
import ml_dtypes

B, S, H = 4, 1024, 4096
NH, NKV, D = 32, 8, 128
THETA = 10000.0
N_CORES = 8
NHL = NH // N_CORES            # 4 local q heads
TOK = B * S                    # 4096 tokens
NT = TOK // 512                # 8 token tiles
KT = H // 128                  # 32 contraction tiles for qkv
QKV_COLS = (NHL + 2) * D       # 768 local qkv columns
WO_K = NHL * D                 # 512 local o_proj contraction
SCALE = 1.0 / float(np.sqrt(D))
WQ_SCALE = 64.0
TRIM = True
POOL_ACC = True


_PROG = {}


def _build_program():
    import concourse.mybir as mybir
    import concourse.bass_isa as bass_isa
    import concourse.tile as tile
    from concourse import bacc

    F32 = mybir.dt.float32
    F32R = mybir.dt.float32r
    BF16 = mybir.dt.bfloat16
    MUL = mybir.AluOpType.mult
    ADD = mybir.AluOpType.add
    EXP = mybir.ActivationFunctionType.Exp
    DR = mybir.MatmulPerfMode.DoubleRow

    nc = bacc.Bacc("TRN2", target_bir_lowering=False, debug=False,
                   num_devices=N_CORES)

    F8 = mybir.dt.float8e4
    hTh_d = nc.dram_tensor("hTh", (H, TOK), F8, kind="ExternalInput")
    hTl_d = nc.dram_tensor("hTl", (H, TOK), F8, kind="ExternalInput")
    wqh_d = nc.dram_tensor("wqh", (H, QKV_COLS), F8, kind="ExternalInput")
    wql_d = nc.dram_tensor("wql", (H, QKV_COLS), F8, kind="ExternalInput")
    woh_d = nc.dram_tensor("woh", (WO_K, H), F8, kind="ExternalInput")
    wol_d = nc.dram_tensor("wol", (WO_K, H), F8, kind="ExternalInput")
    cs_d = nc.dram_tensor("cs", (128, S), BF16, kind="ExternalInput")  # [cos;cos]
    sn_d = nc.dram_tensor("sn", (128, S), BF16, kind="ExternalInput")  # [-sin;sin]
    mk_d = nc.dram_tensor("mk", (4, 128, 512), BF16, kind="ExternalInput")
    on_d = nc.dram_tensor("on", (128, 128), F32R, kind="ExternalInput")
    outT_d = nc.dram_tensor("outT", (H, TOK), BF16, kind="ExternalOutput")

    hTh_r = hTh_d.rearrange("(kp two ki) t -> ki kp two t", ki=128, two=2)
    hTl_r = hTl_d.rearrange("(kp two ki) t -> ki kp two t", ki=128, two=2)
    wqh_r = wqh_d.rearrange("(kp two ki) c -> ki kp two c", ki=128, two=2)
    wql_r = wql_d.rearrange("(kp two ki) c -> ki kp two c", ki=128, two=2)
    woh_r = woh_d.rearrange("(kp two ki) m -> ki kp two m", ki=128, two=2)
    wol_r = wol_d.rearrange("(kp two ki) m -> ki kp two m", ki=128, two=2)
    outT_r = outT_d.rearrange("(mo ki) t -> ki mo t", ki=128)

    with nc.allow_low_precision(reason="bf16 compute within 2e-2 tolerance"), \
         tile.TileContext(nc) as tc:
        with tc.tile_pool(name="persist", bufs=1) as pp, \
             tc.tile_pool(name="io2", bufs=2) as io2, \
             tc.tile_pool(name="sb2", bufs=2) as sb2, \
             tc.tile_pool(name="sb3", bufs=2) as sb3, \
             tc.tile_pool(name="sb1", bufs=1) as sb1, \
             tc.tile_pool(name="ost", bufs=5) as ost, \
             tc.tile_pool(name="probs", bufs=5) as pprob, \
             tc.tile_pool(name="ps_qkv", bufs=2, space="PSUM") as ps_qkv, \
             tc.tile_pool(name="ps_s", bufs=2, space="PSUM") as ps_sp, \
             tc.tile_pool(name="ps_pv", bufs=2, space="PSUM") as ps_pv, \
             tc.tile_pool(name="ps_po", bufs=2, space="PSUM") as ps_po:

            # ---- weights + tables; wq k-groups interleaved with tile-0 hT
            # chunks so the first matmuls start as soon as (wq_g0, hT_c0)
            # land; small tables next (needed ~40us in); wo (needed only
            # from tile 1) last.
            hTh0 = io2.tile([128, 16, 2, 512], F8, tag="hTh")
            hTl0 = io2.tile([128, 16, 2, 512], F8, tag="hTl")
            wq_g = []
            for g in range(4):
                th = pp.tile([128, 4, 2, QKV_COLS], F8, name=f"wqh{g}")
                tl = pp.tile([128, 4, 2, QKV_COLS], F8, name=f"wql{g}")
                gp = slice(g * 4, (g + 1) * 4)
                if g == 0:
                    # v/k weight columns + first hT chunks first, in the
                    # order the 3-term matmul loop consumes them
                    nc.sync.dma_start(th[:, :, :, 512:768],
                                      wqh_r[:, gp, :, 512:768])
                    nc.sync.dma_start(hTh0[:, 0:2, :, :],
                                      hTh_r[:, 0:2, :, 0:512])
                    nc.sync.dma_start(hTl0[:, 0:2, :, :],
                                      hTl_r[:, 0:2, :, 0:512])
                    nc.sync.dma_start(tl[:, :, :, 512:768],
                                      wql_r[:, gp, :, 512:768])
                    nc.sync.dma_start(hTh0[:, 2:4, :, :],
                                      hTh_r[:, 2:4, :, 0:512])
                    nc.sync.dma_start(hTl0[:, 2:4, :, :],
                                      hTl_r[:, 2:4, :, 0:512])
                    nc.sync.dma_start(th[:, :, :, 0:512],
                                      wqh_r[:, gp, :, 0:512])
                    nc.sync.dma_start(tl[:, :, :, 0:512],
                                      wql_r[:, gp, :, 0:512])
                else:
                    nc.sync.dma_start(th[:], wqh_r[:, gp, :, :])
                    nc.sync.dma_start(hTh0[:, gp, :, :],
                                      hTh_r[:, gp, :, 0:512])
                    nc.sync.dma_start(tl[:], wql_r[:, gp, :, :])
                    nc.sync.dma_start(hTl0[:, gp, :, :],
                                      hTl_r[:, gp, :, 0:512])
                wq_g.append((th, tl))
            cs_t = pp.tile([128, S], BF16)
            nc.sync.dma_start(cs_t[:], cs_d[:])
            sn_t = pp.tile([128, S], BF16)
            nc.sync.dma_start(sn_t[:], sn_d[:])
            mk_t = pp.tile([128, 4, 512], BF16)
            nc.sync.dma_start(mk_t[:], mk_d.rearrange("m p f -> p m f"))
            on_t = pp.tile([128, 128], F32R)
            nc.sync.dma_start(on_t[:], on_d[:])
            woh_sb = pp.tile([128, 2, 2, H], F8)
            wol_sb = pp.tile([128, 2, 2, H], F8)

            probs_bufs = [pprob.tile([128, 512], BF16, tag="probs",
                                     name=f"pz{i}") for i in range(5)]
            for t_ in probs_bufs:
                nc.vector.memset(t_[:], 0.0)

            # ---- per-tile state (python-side handles) ----
            kT_t = None      # [128, 1024] bf16, k^T for current batch
            v_t = None       # [128, 8, 128] bf16, v natural for current batch
            prev = None      # (attnT tile, token-tile index) pending o_proj
            carry = None     # tile-1 qkv units pre-built at tile 0
            hT_tiles = {0: (hTh0, hTl0)}

            def emit_oproj_group(state, g, alt=False):
                """One o_proj m-tile: 4 matmuls + evict + maybe DMA."""
                (ah_p, al_p), n_p = state
                mi = g % 2
                if mi == 0:
                    _ostage[0] = ost.tile([128, 2, 512], BF16, tag="ostage",
                                          name="ostage")
                stage = _ostage[0]
                po = ps_po.tile([128, 512], F32, tag="po")
                ms = slice(g * 128, (g + 1) * 128)
                st = True
                for p_ in range(2):
                    hp = slice(2 * p_, 2 * p_ + 2)
                    for wt, at in ((woh_sb, ah_p), (woh_sb, al_p),
                                   (wol_sb, ah_p)):
                        nc.tensor.matmul(
                            po[:], wt[:, p_, :, ms], at[:, hp, :],
                            start=st, stop=(p_ == 1 and wt is wol_sb),
                            perf_mode=DR)
                        st = False
                nc.scalar.mul(stage[:, mi, :], po[:], 1.0 / WQ_SCALE)
                if mi == 1:
                    mg = g // 2
                    nc.sync.dma_start(
                        outT_r[:, mg * 2:(mg + 1) * 2,
                               n_p * 512:(n_p + 1) * 512], stage[:])

            _ostage = [None]

            for n in range(NT):
                b, half = n // 2, n % 2
                csl = cs_t[:, half * 512:(half + 1) * 512]
                snl = sn_t[:, half * 512:(half + 1) * 512]

                # queue of pending o_proj groups for tile n-1
                po_queue = list(range(32)) if prev is not None else []
                po_state = prev
                po_cap = [0]

                def drain_po(k=1, force=False):
                    for _ in range(k):
                        if po_queue and (force or 32 - len(po_queue) < po_cap[0]):
                            emit_oproj_group(po_state, po_queue.pop(0))
                        elif fill_q:
                            fill_q.pop(0)()

                # ---- prefetch next tile's hT (double-buffered) ----
                if n + 1 < NT:
                    nxh = io2.tile([128, 16, 2, 512], F8, tag="hTh",
                                   name="hTnh")
                    nxl = io2.tile([128, 16, 2, 512], F8, tag="hTl",
                                   name="hTnl")
                    tsl = slice((n + 1) * 512, (n + 2) * 512)
                    for g in range(4):
                        gp = slice(g * 4, (g + 1) * 4)
                        nc.sync.dma_start(nxh[:, gp, :, :],
                                          hTh_r[:, gp, :, tsl])
                        nc.sync.dma_start(nxl[:, gp, :, :],
                                          hTl_r[:, gp, :, tsl])
                    hT_tiles[n + 1] = (nxh, nxl)
                hTh_t, hTl_t = hT_tiles.pop(n)

                # ---- qkv + RoPE ----
                fill_q = []
                if n == 1 and carry:
                    qT_t, kT_t, v_t = carry["qkv_out"]
                else:
                    qT_t = sb2.tile([128, NHL, 512], BF16, tag="qT")
                    if half == 0:
                        kT_t = sb2.tile([128, S], BF16, tag="kT")
                        v_t = sb2.tile([128, 8, 128], BF16, tag="v")

                def qkv_epilogue(m, ps):
                    if m < NHL or m == 4:
                        # RoPE: out = ps*[cos;cos] + rot(ps)*[-sin;sin]
                        if m < NHL:
                            out = qT_t[:, m, :]
                        else:
                            out = kT_t[:, half * 512:(half + 1) * 512]
                        tco = sb1.tile([128, 512], BF16, tag="tco")
                        tsi = sb1.tile([128, 512], BF16, tag="tsi")
                        nc.vector.tensor_tensor(tco[:], ps[:], csl, op=MUL)
                        nc.vector.tensor_tensor(tsi[0:64, :], ps[64:128, :],
                                                snl[0:64, :], op=MUL)
                        nc.vector.tensor_tensor(tsi[64:128, :], ps[0:64, :],
                                                snl[64:128, :], op=MUL)
                        nc.vector.tensor_tensor(out, tco[:], tsi[:], op=ADD)
                    else:
                        # v: evict bf16, then xbar DMA-transpose to [tok, d]
                        vT_tmp = sb1.tile([128, 512], BF16, tag="vT")
                        nc.scalar.mul(vT_tmp[:], ps[:], 1.0 / WQ_SCALE)
                        for c4 in range(4):
                            nc.sync.dma_start(
                                v_t[:, half * 4 + c4, :],
                                vT_tmp[:, c4 * 128:(c4 + 1) * 128],
                                transpose=True)

                M_ORDER = (5, 4, 0, 1, 2, 3)  # v,k first: their consumers
                # sit at the head of the attention phase
                if n == 0:
                    # startup: g-outer so matmuls start as DMA chunks land;
                    # 6 concurrent banks borrowed from the idle s/pv pools
                    banks = {5: ps_qkv.tile([128, 512], F32, tag="qkv", name="b5"),
                             4: ps_qkv.tile([128, 512], F32, tag="qkv", name="b4"),
                             0: ps_sp.tile([128, 512], F32, tag="s", name="b0"),
                             1: ps_sp.tile([128, 512], F32, tag="s", name="b1"),
                             2: ps_pv.tile([128, 512], F32, tag="pv", name="b2"),
                             3: ps_pv.tile([128, 512], F32, tag="pv", name="b3")}
                    for g in range(4):
                        th, tl = wq_g[g]
                        for m in M_ORDER:
                            ms = slice(m * 128, (m + 1) * 128)
                            for kk in range(4):
                                kp = g * 4 + kk
                                st = kp == 0
                                for wt, ht in ((th, hTh_t), (th, hTl_t),
                                               (tl, hTh_t)):
                                    nc.tensor.matmul(
                                        banks[m][:], wt[:, kk, :, ms],
                                        ht[:, kp, :, :], start=st,
                                        stop=(kp == 15 and wt is tl),
                                        perf_mode=DR)
                                    st = False
                    for m in M_ORDER:
                        qkv_epilogue(m, banks[m])
                elif n == 1 and carry:
                    rest = carry["units"]
                    while rest:
                        rest.pop(0)()
                    carry = None
                else:
                    for m in M_ORDER:
                        ms = slice(m * 128, (m + 1) * 128)
                        ps = ps_qkv.tile([128, 512], F32, tag="qkv")
                        for kp in range(16):
                            th, tl = wq_g[kp // 4]
                            kk = kp % 4
                            st = kp == 0
                            for wt, ht in ((th, hTh_t), (th, hTl_t),
                                           (tl, hTh_t)):
                                nc.tensor.matmul(
                                    ps[:], wt[:, kk, :, ms],
                                    ht[:, kp, :, :], start=st,
                                    stop=(kp == 15 and wt is tl),
                                    perf_mode=DR)
                                st = False
                        qkv_epilogue(m, ps)

                if n == 0:
                    # pre-build tile-1 qkv units; drain into tile-0
                    # attention gaps (no o_proj exists yet)
                    n1h, n1l = hT_tiles[1]
                    qT1 = sb2.tile([128, NHL, 512], BF16, tag="qT",
                                   name="qT1")
                    cs1 = cs_t[:, 512:1024]
                    sn1 = sn_t[:, 512:1024]
                    st1 = {}

                    def mk_unit(m, kp):
                        def emit():
                            ms = slice(m * 128, (m + 1) * 128)
                            if kp == 0:
                                st1[m] = ps_qkv.tile([128, 512], F32,
                                                     tag="qkv", name="q1")
                            ps = st1[m]
                            th, tl = wq_g[kp // 4]
                            kk = kp % 4
                            st = kp == 0
                            for wt, ht in ((th, n1h), (th, n1l), (tl, n1h)):
                                nc.tensor.matmul(
                                    ps[:], wt[:, kk, :, ms],
                                    ht[:, kp, :, :], start=st,
                                    stop=(kp == 15 and wt is tl),
                                    perf_mode=DR)
                                st = False
                            if kp == 15:
                                if m < NHL:
                                    out = qT1[:, m, :]
                                else:
                                    out = kT_t[:, 512:1024]
                                if m < NHL or m == 4:
                                    tco = sb1.tile([128, 512], BF16,
                                                   tag="tco", name="tc1")
                                    tsi = sb1.tile([128, 512], BF16,
                                                   tag="tsi", name="ts1")
                                    nc.vector.tensor_tensor(
                                        tco[:], ps[:], cs1, op=MUL)
                                    nc.vector.tensor_tensor(
                                        tsi[0:64, :], ps[64:128, :],
                                        sn1[0:64, :], op=MUL)
                                    nc.vector.tensor_tensor(
                                        tsi[64:128, :], ps[0:64, :],
                                        sn1[64:128, :], op=MUL)
                                    nc.vector.tensor_tensor(
                                        out, tco[:], tsi[:], op=ADD)
                                else:
                                    vT_tmp = sb1.tile([128, 512], BF16,
                                                      tag="vT", name="v1")
                                    nc.scalar.mul(vT_tmp[:], ps[:],
                                                  1.0 / WQ_SCALE)
                                    for c4 in range(4):
                                        nc.sync.dma_start(
                                            v_t[:, 4 + c4, :],
                                            vT_tmp[:, c4 * 128:
                                                   (c4 + 1) * 128],
                                            transpose=True)
                        return emit

                    units = [mk_unit(m, kp) for m in M_ORDER
                             for kp in range(16)]
                    fill_q = units
                    carry = {"units": units,
                             "qkv_out": (qT1, kT_t, v_t)}
                    # wo loads behind the tile-1 hT prefetch; first o_proj
                    # consumer is a full tile away
                    nc.sync.dma_start(woh_sb[:], woh_r[:])
                    nc.sync.dma_start(wol_sb[:], wol_r[:])

                # ---- attention (+ interleaved o_proj of tile n-1) ----
                jmax = 4 + half * 4
                at_h = sb2.tile([128, NHL, 512], F8, tag="at_h")
                at_l = sb2.tile([128, NHL, 512], F8, tag="at_l")

                def den_chain(h, acc, pv):
                    # softmax denominator (partition all-reduce broadcasts
                    # the column sum to every partition) / normalization
                    dbc = sb1.tile([128, 512], F32R, tag="dbc", name="dbc")
                    nc.gpsimd.partition_all_reduce(
                        dbc[:], acc[:], 128, reduce_op=bass_isa.ReduceOp.add)
                    drain_po(2)
                    rbc = sb1.tile([128, 512], F32R, tag="rbc", name="rbc")
                    nc.vector.reciprocal(rbc[:], dbc[:])
                    at_t = sb1.tile([128, 512], BF16, tag="at_t",
                                    name="at_t")
                    nc.vector.tensor_tensor(at_t[:], pv[:], rbc[:], op=MUL)
                    nc.vector.tensor_copy(at_h[:, h, :], at_t[:])
                    nc.vector.tensor_tensor(at_l[:, h, :], at_t[:],
                                            at_h[:, h, :],
                                            op=mybir.AluOpType.subtract)
                    drain_po(2)

                pending_den = None
                for h in range(NHL):
                    po_cap[0] = 8 * (h + 1) if h < NHL - 1 else (29 if half else 26)
                    acc = sb2.tile([128, 512], F32R, tag="acc")
                    pv = ps_pv.tile([128, 512], F32, tag="pv")
                    pj = []  # pending probs for PV (lag 2)
                    for j in range(jmax):
                        mf0 = max(0, j * 128 - half * 512)
                        f0 = mf0 if TRIM else 0
                        w = 512 - f0
                        sps = ps_sp.tile([128, 512], F32, tag="s")
                        nc.tensor.matmul(
                            sps[:, 0:w], kT_t[:, j * 128:(j + 1) * 128],
                            qT_t[:, h, f0:512], start=True, stop=True)
                        probs = pprob.tile([128, 512], BF16, tag="probs")
                        nc.scalar.activation(probs[:, f0:512], sps[:, 0:w],
                                             EXP, scale=SCALE)
                        if j * 128 >= half * 512:
                            # diagonal block: full-width mask zeroes
                            # probs[:, :mf0] (stale) and the upper triangle
                            nc.vector.tensor_tensor(
                                probs[:], probs[:],
                                mk_t[:, j - half * 4, :], op=MUL)
                        acc_eng = (nc.gpsimd if POOL_ACC and h < NHL - 1
                                   else nc.vector)
                        if j == 0:
                            acc_eng.tensor_copy(acc[:], probs[:])
                        else:
                            acc_eng.tensor_tensor(
                                acc[:, mf0:512], acc[:, mf0:512],
                                probs[:, mf0:512], op=ADD)
                        pj.append((j, probs))
                        drain_po(1)
                        if len(pj) >= 4:
                            jj, pp_ = pj.pop(0)
                            nc.tensor.matmul(
                                pv[:], v_t[:, jj, :], pp_[:],
                                start=(jj == 0), stop=False)
                            drain_po(1)
                        if j == 3 and pending_den is not None:
                            den_chain(*pending_den)
                            pending_den = None
                    for (jj, pp_) in pj:
                        nc.tensor.matmul(
                            pv[:], v_t[:, jj, :], pp_[:],
                            start=(jj == 0), stop=(jj == jmax - 1))
                        drain_po(1)
                    pending_den = (h, acc, pv)
                if n == NT - 1:
                    den_chain(*pending_den)
                    drain_po(32, force=True)
                else:
                    drain_po(3, force=True)
                    den_chain(*pending_den)
                    drain_po(32, force=True)
                prev = ((at_h, at_l), n)

            # epilogue: o_proj for the last tile
            po_queue = list(range(32))
            po_state = prev
            while po_queue:
                emit_oproj_group(po_state, po_queue.pop(0), alt=True)

    nc.compile()
    return nc


def _get_program():
    if "nc" not in _PROG:
        _PROG["nc"] = _build_program()
    return _PROG["nc"]


def _host_inputs(positions, hidden_states, w_qkv, w_o):
    positions = np.asarray(positions)
    hidden_states = np.asarray(hidden_states, dtype=np.float32)
    w_qkv = np.asarray(w_qkv, dtype=np.float32)
    w_o = np.asarray(w_o, dtype=np.float32)

    hT = np.ascontiguousarray(hidden_states.reshape(TOK, H).T)
    hTh = hT.astype(ml_dtypes.float8_e4m3)
    hTl = (hT - hTh.astype(np.float32)).astype(ml_dtypes.float8_e4m3)

    pos0 = positions[0].astype(np.float32)
    inv = 1.0 / (THETA ** (np.arange(64, dtype=np.float32) / 64.0))
    ang = inv[:, None] * pos0[None, :]            # [64, S]
    c = np.cos(ang).astype(np.float32) / WQ_SCALE
    s = np.sin(ang).astype(np.float32) / WQ_SCALE
    cs = np.concatenate([c, c], axis=0).astype(ml_dtypes.bfloat16)
    sn = np.concatenate([-s, s], axis=0).astype(ml_dtypes.bfloat16)

    p = np.arange(128)[:, None]
    f = np.arange(512)[None, :]
    mk = np.zeros((4, 128, 512), dtype=ml_dtypes.bfloat16)
    for mi in range(4):
        mk[mi] = (p + 128 * mi <= f).astype(ml_dtypes.bfloat16)
    ones = np.ones((128, 128), dtype=np.float32)

    in_maps = []
    for c_ in range(N_CORES):
        q0 = c_ * NHL * D
        kc = NH * D + c_ * D
        vc = NH * D + NKV * D + c_ * D
        wq = np.ascontiguousarray(np.concatenate(
            [w_qkv[:, q0:q0 + NHL * D],
             w_qkv[:, kc:kc + D],
             w_qkv[:, vc:vc + D]], axis=1)) * WQ_SCALE
        wqh = wq.astype(ml_dtypes.float8_e4m3)
        wql = (wq - wqh.astype(np.float32)).astype(ml_dtypes.float8_e4m3)
        wo = np.ascontiguousarray(
            w_o[c_ * WO_K:(c_ + 1) * WO_K, :]) * WQ_SCALE
        woh = wo.astype(ml_dtypes.float8_e4m3)
        wol = (wo - woh.astype(np.float32)).astype(ml_dtypes.float8_e4m3)
        in_maps.append({"hTh": hTh, "hTl": hTl, "wqh": wqh,
                        "wql": wql, "woh": woh, "wol": wol, "cs": cs, "sn": sn,
                        "mk": mk, "on": ones})
    return in_maps


def run(positions, hidden_states, w_qkv, w_o, trace=False):
    from concourse import bass_utils
    nc = _get_program()
    in_maps = _host_inputs(positions, hidden_states, w_qkv, w_o)
    res = bass_utils.run_bass_kernel_spmd(
        nc, in_maps, core_ids=list(range(N_CORES)), trace=trace)
    acc = np.zeros((H, TOK), dtype=np.float32)
    for c in range(N_CORES):
        acc += np.asarray(res.results[c]["outT"], dtype=np.float32)
    out = np.ascontiguousarray(acc.T).reshape(B, S, H)
    return out, res


def kernel(positions, hidden_states, w_qkv, w_o):
    out, _ = run(positions, hidden_states, w_qkv, w_o, trace=False)
    return out



# revision 13
# speedup vs baseline: 1.2262x; 1.2262x over previous
"""Llama GQA attention (B=4,S=1024,H=4096,NH=32,NKV=8,D=128) on 8 TRN2 cores.

Strategy: tensor-parallel over heads (4 q heads + 1 kv head per core), host
all-reduce of o_proj partials.  v2: single fused pipeline tuned for PE
occupancy against the TimelineSim cost model.

Per 512-token tile n (batch b = n//2, half = n%2):
  1. qkv^T matmuls (bf16, m-outer k-inner, 2 rotating PSUM banks)
  2. RoPE epilogues per m as banks complete (DVE for q heads, Act for k;
     4-op rotate-half form using stacked [cos;cos] / [-sin;sin] tables)
  3. causal attention per head with column-trimmed score/PV matmuls,
     exp on Act (bf16 probs), diagonal-block mask on DVE, softmax
     denominator accumulated on the (otherwise idle) Pool engine
  4. o_proj matmuls of tile n-1 interleaved between attention matmuls to
     hide exp/mask latency; evictions rotate over Act/DVE/Pool; bf16
     output staged and DMA'd in 4-m-tile batches.
"""

import numpy as np
import ml_dtypes

B, S, H = 4, 1024, 4096
NH, NKV, D = 32, 8, 128
THETA = 10000.0
N_CORES = 8
NHL = NH // N_CORES            # 4 local q heads
TOK = B * S                    # 4096 tokens
NT = TOK // 512                # 8 token tiles
KT = H // 128                  # 32 contraction tiles for qkv
QKV_COLS = (NHL + 2) * D       # 768 local qkv columns
WO_K = NHL * D                 # 512 local o_proj contraction
SCALE = 1.0 / float(np.sqrt(D))
WQ_SCALE = 64.0
TRIM = True
POOL_ACC = True
# Karatsuba-gamma 2-slot fp8 decomposition: x*w ~ (1-1/G) * [x1@w1 +
# (x1+G*x0)@Q((w1+G*w0)/(G-1))].  The 1/(G-1) folds into the host-side
# quantization of the t-weights; the global (1-1/G) folds into the RoPE
# tables / v-evict scale (qkv) and the host reduction (o_proj).
GQ = 6.0                       # gamma for qkv projection
GO = 6.0                       # gamma for o_proj
QC = 1.0 - 1.0 / GQ
OC = 1.0 - 1.0 / GO


_PROG = {}


def _build_program():
    import concourse.mybir as mybir
    import concourse.bass_isa as bass_isa
    import concourse.tile as tile
    from concourse import bacc

    F32 = mybir.dt.float32
    F32R = mybir.dt.float32r
    BF16 = mybir.dt.bfloat16
    MUL = mybir.AluOpType.mult
    ADD = mybir.AluOpType.add
    EXP = mybir.ActivationFunctionType.Exp
    DR = mybir.MatmulPerfMode.DoubleRow

    nc = bacc.Bacc("TRN2", target_bir_lowering=False, debug=False,
                   num_devices=N_CORES)

    F8 = mybir.dt.float8e4
    hTh_d = nc.dram_tensor("hTh", (H, TOK), F8, kind="ExternalInput")
    hTl_d = nc.dram_tensor("hTl", (H, TOK), F8, kind="ExternalInput")
    wqh_d = nc.dram_tensor("wqh", (H, QKV_COLS), F8, kind="ExternalInput")
    wql_d = nc.dram_tensor("wql", (H, QKV_COLS), F8, kind="ExternalInput")
    woh_d = nc.dram_tensor("woh", (WO_K, H), F8, kind="ExternalInput")
    wol_d = nc.dram_tensor("wol", (WO_K, H), F8, kind="ExternalInput")
    cs_d = nc.dram_tensor("cs", (128, S), BF16, kind="ExternalInput")  # [cos;cos]
    sn_d = nc.dram_tensor("sn", (128, S), BF16, kind="ExternalInput")  # [-sin;sin]
    mk_d = nc.dram_tensor("mk", (4, 128, 512), BF16, kind="ExternalInput")
    on_d = nc.dram_tensor("on", (128, 128), F32R, kind="ExternalInput")
    outT_d = nc.dram_tensor("outT", (H, TOK), BF16, kind="ExternalOutput")

    hTh_r = hTh_d.rearrange("(kp two ki) t -> ki kp two t", ki=128, two=2)
    hTl_r = hTl_d.rearrange("(kp two ki) t -> ki kp two t", ki=128, two=2)
    wqh_r = wqh_d.rearrange("(kp two ki) c -> ki kp two c", ki=128, two=2)
    wql_r = wql_d.rearrange("(kp two ki) c -> ki kp two c", ki=128, two=2)
    woh_r = woh_d.rearrange("(kp two ki) m -> ki kp two m", ki=128, two=2)
    wol_r = wol_d.rearrange("(kp two ki) m -> ki kp two m", ki=128, two=2)
    outT_r = outT_d.rearrange("(mo ki) t -> ki mo t", ki=128)

    with nc.allow_low_precision(reason="bf16 compute within 2e-2 tolerance"), \
         tile.TileContext(nc) as tc:
        with tc.tile_pool(name="persist", bufs=1) as pp, \
             tc.tile_pool(name="io2", bufs=2) as io2, \
             tc.tile_pool(name="sb2", bufs=2) as sb2, \
             tc.tile_pool(name="sb3", bufs=2) as sb3, \
             tc.tile_pool(name="sb1", bufs=1) as sb1, \
             tc.tile_pool(name="ost", bufs=5) as ost, \
             tc.tile_pool(name="probs", bufs=5) as pprob, \
             tc.tile_pool(name="ps_qkv", bufs=2, space="PSUM") as ps_qkv, \
             tc.tile_pool(name="ps_s", bufs=2, space="PSUM") as ps_sp, \
             tc.tile_pool(name="ps_pv", bufs=2, space="PSUM") as ps_pv, \
             tc.tile_pool(name="ps_po", bufs=2, space="PSUM") as ps_po:

            # ---- weights + tables; wq k-groups interleaved with tile-0 hT
            # chunks so the first matmuls start as soon as (wq_g0, hT_c0)
            # land; small tables next (needed ~40us in); wo (needed only
            # from tile 1) last.
            hTh0 = io2.tile([128, 16, 2, 512], F8, tag="hTh")
            hTl0 = io2.tile([128, 16, 2, 512], F8, tag="hTl")
            wq_g = []
            for g in range(4):
                th = pp.tile([128, 4, 2, QKV_COLS], F8, name=f"wqh{g}")
                tl = pp.tile([128, 4, 2, QKV_COLS], F8, name=f"wql{g}")
                gp = slice(g * 4, (g + 1) * 4)
                if g == 0:
                    # v/k weight columns + first hT chunks first, in the
                    # order the 3-term matmul loop consumes them
                    nc.sync.dma_start(th[:, :, :, 512:768],
                                      wqh_r[:, gp, :, 512:768])
                    nc.sync.dma_start(hTh0[:, 0:2, :, :],
                                      hTh_r[:, 0:2, :, 0:512])
                    nc.sync.dma_start(hTl0[:, 0:2, :, :],
                                      hTl_r[:, 0:2, :, 0:512])
                    nc.sync.dma_start(tl[:, :, :, 512:768],
                                      wql_r[:, gp, :, 512:768])
                    nc.sync.dma_start(hTh0[:, 2:4, :, :],
                                      hTh_r[:, 2:4, :, 0:512])
                    nc.sync.dma_start(hTl0[:, 2:4, :, :],
                                      hTl_r[:, 2:4, :, 0:512])
                    nc.sync.dma_start(th[:, :, :, 0:512],
                                      wqh_r[:, gp, :, 0:512])
                    nc.sync.dma_start(tl[:, :, :, 0:512],
                                      wql_r[:, gp, :, 0:512])
                else:
                    nc.sync.dma_start(th[:], wqh_r[:, gp, :, :])
                    nc.sync.dma_start(hTh0[:, gp, :, :],
                                      hTh_r[:, gp, :, 0:512])
                    nc.sync.dma_start(tl[:], wql_r[:, gp, :, :])
                    nc.sync.dma_start(hTl0[:, gp, :, :],
                                      hTl_r[:, gp, :, 0:512])
                wq_g.append((th, tl))
            cs_t = pp.tile([128, S], BF16)
            nc.sync.dma_start(cs_t[:], cs_d[:])
            sn_t = pp.tile([128, S], BF16)
            nc.sync.dma_start(sn_t[:], sn_d[:])
            mk_t = pp.tile([128, 4, 512], BF16)
            nc.sync.dma_start(mk_t[:], mk_d.rearrange("m p f -> p m f"))
            on_t = pp.tile([128, 128], F32R)
            nc.sync.dma_start(on_t[:], on_d[:])
            woh_sb = pp.tile([128, 2, 2, H], F8)
            wol_sb = pp.tile([128, 2, 2, H], F8)

            probs_bufs = [pprob.tile([128, 512], BF16, tag="probs",
                                     name=f"pz{i}") for i in range(5)]
            for t_ in probs_bufs:
                nc.vector.memset(t_[:], 0.0)

            # ---- per-tile state (python-side handles) ----
            kT_t = None      # [128, 1024] bf16, k^T for current batch
            v_t = None       # [128, 8, 128] bf16, v natural for current batch
            prev = None      # (attnT tile, token-tile index) pending o_proj
            carry = None     # tile-1 qkv units pre-built at tile 0
            hT_tiles = {0: (hTh0, hTl0)}

            def emit_oproj_group(state, g, alt=False):
                """One o_proj m-tile: 4 matmuls + evict + maybe DMA."""
                (ah_p, al_p), n_p = state
                mi = g % 2
                if mi == 0:
                    _ostage[0] = ost.tile([128, 2, 512], BF16, tag="ostage",
                                          name="ostage")
                stage = _ostage[0]
                po = ps_po.tile([128, 512], F32, tag="po")
                ms = slice(g * 128, (g + 1) * 128)
                st = True
                for p_ in range(2):
                    hp = slice(2 * p_, 2 * p_ + 2)
                    for wt, at in ((woh_sb, ah_p), (wol_sb, al_p)):
                        nc.tensor.matmul(
                            po[:], wt[:, p_, :, ms], at[:, hp, :],
                            start=st, stop=(p_ == 1 and wt is wol_sb),
                            perf_mode=DR)
                        st = False
                nc.scalar.mul(stage[:, mi, :], po[:], 1.0 / WQ_SCALE)
                if mi == 1:
                    mg = g // 2
                    nc.sync.dma_start(
                        outT_r[:, mg * 2:(mg + 1) * 2,
                               n_p * 512:(n_p + 1) * 512], stage[:])

            _ostage = [None]

            for n in range(NT):
                b, half = n // 2, n % 2
                csl = cs_t[:, half * 512:(half + 1) * 512]
                snl = sn_t[:, half * 512:(half + 1) * 512]

                # queue of pending o_proj groups for tile n-1
                po_queue = list(range(32)) if prev is not None else []
                po_state = prev
                po_cap = [0]

                def drain_po(k=1, force=False):
                    for _ in range(k):
                        if po_queue and (force or 32 - len(po_queue) < po_cap[0]):
                            emit_oproj_group(po_state, po_queue.pop(0))
                        elif fill_q:
                            fill_q.pop(0)()

                # ---- prefetch next tile's hT (double-buffered) ----
                if n + 1 < NT:
                    nxh = io2.tile([128, 16, 2, 512], F8, tag="hTh",
                                   name="hTnh")
                    nxl = io2.tile([128, 16, 2, 512], F8, tag="hTl",
                                   name="hTnl")
                    tsl = slice((n + 1) * 512, (n + 2) * 512)
                    for g in range(4):
                        gp = slice(g * 4, (g + 1) * 4)
                        nc.sync.dma_start(nxh[:, gp, :, :],
                                          hTh_r[:, gp, :, tsl])
                        nc.sync.dma_start(nxl[:, gp, :, :],
                                          hTl_r[:, gp, :, tsl])
                    hT_tiles[n + 1] = (nxh, nxl)
                hTh_t, hTl_t = hT_tiles.pop(n)

                # ---- qkv + RoPE ----
                fill_q = []
                if n == 1 and carry:
                    qT_t, kT_t, v_t = carry["qkv_out"]
                else:
                    qT_t = sb2.tile([128, NHL, 512], BF16, tag="qT")
                    if half == 0:
                        kT_t = sb2.tile([128, S], BF16, tag="kT")
                        v_t = sb2.tile([128, 8, 128], BF16, tag="v")

                def qkv_epilogue(m, ps):
                    if m < NHL or m == 4:
                        # RoPE: out = ps*[cos;cos] + rot(ps)*[-sin;sin]
                        if m < NHL:
                            out = qT_t[:, m, :]
                        else:
                            out = kT_t[:, half * 512:(half + 1) * 512]
                        tco = sb1.tile([128, 512], BF16, tag="tco")
                        tsi = sb1.tile([128, 512], BF16, tag="tsi")
                        nc.vector.tensor_tensor(tco[:], ps[:], csl, op=MUL)
                        nc.vector.tensor_tensor(tsi[0:64, :], ps[64:128, :],
                                                snl[0:64, :], op=MUL)
                        nc.vector.tensor_tensor(tsi[64:128, :], ps[0:64, :],
                                                snl[64:128, :], op=MUL)
                        nc.vector.tensor_tensor(out, tco[:], tsi[:], op=ADD)
                    else:
                        # v: evict bf16, then xbar DMA-transpose to [tok, d]
                        vT_tmp = sb1.tile([128, 512], BF16, tag="vT")
                        nc.scalar.mul(vT_tmp[:], ps[:], QC / WQ_SCALE)
                        for c4 in range(4):
                            nc.sync.dma_start(
                                v_t[:, half * 4 + c4, :],
                                vT_tmp[:, c4 * 128:(c4 + 1) * 128],
                                transpose=True)

                M_ORDER = (5, 4, 0, 1, 2, 3)  # v,k first: their consumers
                # sit at the head of the attention phase
                if n == 0:
                    # startup: g-outer so matmuls start as DMA chunks land;
                    # 6 concurrent banks borrowed from the idle s/pv pools
                    banks = {5: ps_qkv.tile([128, 512], F32, tag="qkv", name="b5"),
                             4: ps_qkv.tile([128, 512], F32, tag="qkv", name="b4"),
                             0: ps_sp.tile([128, 512], F32, tag="s", name="b0"),
                             1: ps_sp.tile([128, 512], F32, tag="s", name="b1"),
                             2: ps_pv.tile([128, 512], F32, tag="pv", name="b2"),
                             3: ps_pv.tile([128, 512], F32, tag="pv", name="b3")}
                    for g in range(4):
                        th, tl = wq_g[g]
                        for m in M_ORDER:
                            ms = slice(m * 128, (m + 1) * 128)
                            for kk in range(4):
                                kp = g * 4 + kk
                                st = kp == 0
                                for wt, ht in ((th, hTh_t), (tl, hTl_t)):
                                    nc.tensor.matmul(
                                        banks[m][:], wt[:, kk, :, ms],
                                        ht[:, kp, :, :], start=st,
                                        stop=(kp == 15 and wt is tl),
                                        perf_mode=DR)
                                    st = False
                    for m in M_ORDER:
                        qkv_epilogue(m, banks[m])
                elif n == 1 and carry:
                    rest = carry["units"]
                    while rest:
                        rest.pop(0)()
                    carry = None
                else:
                    for m in M_ORDER:
                        ms = slice(m * 128, (m + 1) * 128)
                        ps = ps_qkv.tile([128, 512], F32, tag="qkv")
                        for kp in range(16):
                            th, tl = wq_g[kp // 4]
                            kk = kp % 4
                            st = kp == 0
                            for wt, ht in ((th, hTh_t), (tl, hTl_t)):
                                nc.tensor.matmul(
                                    ps[:], wt[:, kk, :, ms],
                                    ht[:, kp, :, :], start=st,
                                    stop=(kp == 15 and wt is tl),
                                    perf_mode=DR)
                                st = False
                        qkv_epilogue(m, ps)

                if n == 0:
                    # pre-build tile-1 qkv units; drain into tile-0
                    # attention gaps (no o_proj exists yet)
                    n1h, n1l = hT_tiles[1]
                    qT1 = sb2.tile([128, NHL, 512], BF16, tag="qT",
                                   name="qT1")
                    cs1 = cs_t[:, 512:1024]
                    sn1 = sn_t[:, 512:1024]
                    st1 = {}

                    def mk_unit(m, kp):
                        def emit():
                            ms = slice(m * 128, (m + 1) * 128)
                            if kp == 0:
                                st1[m] = ps_qkv.tile([128, 512], F32,
                                                     tag="qkv", name="q1")
                            ps = st1[m]
                            th, tl = wq_g[kp // 4]
                            kk = kp % 4
                            st = kp == 0
                            for wt, ht in ((th, n1h), (tl, n1l)):
                                nc.tensor.matmul(
                                    ps[:], wt[:, kk, :, ms],
                                    ht[:, kp, :, :], start=st,
                                    stop=(kp == 15 and wt is tl),
                                    perf_mode=DR)
                                st = False
                            if kp == 15:
                                if m < NHL:
                                    out = qT1[:, m, :]
                                else:
                                    out = kT_t[:, 512:1024]
                                if m < NHL or m == 4:
                                    tco = sb1.tile([128, 512], BF16,
                                                   tag="tco", name="tc1")
                                    tsi = sb1.tile([128, 512], BF16,
                                                   tag="tsi", name="ts1")
                                    nc.vector.tensor_tensor(
                                        tco[:], ps[:], cs1, op=MUL)
                                    nc.vector.tensor_tensor(
                                        tsi[0:64, :], ps[64:128, :],
                                        sn1[0:64, :], op=MUL)
                                    nc.vector.tensor_tensor(
                                        tsi[64:128, :], ps[0:64, :],
                                        sn1[64:128, :], op=MUL)
                                    nc.vector.tensor_tensor(
                                        out, tco[:], tsi[:], op=ADD)
                                else:
                                    vT_tmp = sb1.tile([128, 512], BF16,
                                                      tag="vT", name="v1")
                                    nc.scalar.mul(vT_tmp[:], ps[:],
                                                  QC / WQ_SCALE)
                                    for c4 in range(4):
                                        nc.sync.dma_start(
                                            v_t[:, 4 + c4, :],
                                            vT_tmp[:, c4 * 128:
                                                   (c4 + 1) * 128],
                                            transpose=True)
                        return emit

                    units = [mk_unit(m, kp) for m in M_ORDER
                             for kp in range(16)]
                    fill_q = units
                    carry = {"units": units,
                             "qkv_out": (qT1, kT_t, v_t)}
                    # wo loads behind the tile-1 hT prefetch; first o_proj
                    # consumer is a full tile away
                    nc.sync.dma_start(woh_sb[:], woh_r[:])
                    nc.sync.dma_start(wol_sb[:], wol_r[:])

                # ---- attention (+ interleaved o_proj of tile n-1) ----
                jmax = 4 + half * 4
                at_h = sb2.tile([128, NHL, 512], F8, tag="at_h")
                at_l = sb2.tile([128, NHL, 512], F8, tag="at_l")

                def den_chain(h, acc, pv):
                    # softmax denominator (partition all-reduce broadcasts
                    # the column sum to every partition) / normalization
                    dbc = sb1.tile([128, 512], F32R, tag="dbc", name="dbc")
                    nc.gpsimd.partition_all_reduce(
                        dbc[:], acc[:], 128, reduce_op=bass_isa.ReduceOp.add)
                    drain_po(2)
                    rbc = sb1.tile([128, 512], F32R, tag="rbc", name="rbc")
                    nc.vector.reciprocal(rbc[:], dbc[:])
                    at_t = sb1.tile([128, 512], BF16, tag="at_t",
                                    name="at_t")
                    nc.vector.tensor_tensor(at_t[:], pv[:], rbc[:], op=MUL)
                    nc.vector.tensor_copy(at_h[:, h, :], at_t[:])
                    # t_a = at_h + GO*(at_t - at_h), fp8 (Karatsuba t-term)
                    tml = sb1.tile([128, 512], BF16, tag="tml", name="tml")
                    nc.vector.tensor_tensor(tml[:], at_t[:], at_h[:, h, :],
                                            op=mybir.AluOpType.subtract)
                    drain_po(1)
                    tmg = sb1.tile([128, 512], BF16, tag="tmg", name="tmg")
                    nc.vector.tensor_scalar(tmg[:], tml[:], GO, None, op0=MUL)
                    nc.vector.tensor_tensor(at_l[:, h, :], at_h[:, h, :],
                                            tmg[:], op=ADD)
                    drain_po(2)

                pending_den = None
                for h in range(NHL):
                    po_cap[0] = 8 * (h + 1) if h < NHL - 1 else (29 if half else 26)
                    acc = sb2.tile([128, 512], F32R, tag="acc")
                    pv = ps_pv.tile([128, 512], F32, tag="pv")
                    pj = []  # pending probs for PV (lag 2)
                    for j in range(jmax):
                        mf0 = max(0, j * 128 - half * 512)
                        f0 = mf0 if TRIM else 0
                        w = 512 - f0
                        sps = ps_sp.tile([128, 512], F32, tag="s")
                        nc.tensor.matmul(
                            sps[:, 0:w], kT_t[:, j * 128:(j + 1) * 128],
                            qT_t[:, h, f0:512], start=True, stop=True)
                        probs = pprob.tile([128, 512], BF16, tag="probs")
                        nc.scalar.activation(probs[:, f0:512], sps[:, 0:w],
                                             EXP, scale=SCALE)
                        if j * 128 >= half * 512:
                            # diagonal block: full-width mask zeroes
                            # probs[:, :mf0] (stale) and the upper triangle
                            nc.vector.tensor_tensor(
                                probs[:], probs[:],
                                mk_t[:, j - half * 4, :], op=MUL)
                        acc_eng = (nc.gpsimd if POOL_ACC and h < NHL - 1
                                   else nc.vector)
                        if j == 0:
                            acc_eng.tensor_copy(acc[:], probs[:])
                        else:
                            acc_eng.tensor_tensor(
                                acc[:, mf0:512], acc[:, mf0:512],
                                probs[:, mf0:512], op=ADD)
                        pj.append((j, probs))
                        drain_po(1)
                        if len(pj) >= 4:
                            jj, pp_ = pj.pop(0)
                            g0 = max(0, jj * 128 - half * 512) if TRIM else 0
                            nc.tensor.matmul(
                                pv[:, g0:512], v_t[:, jj, :], pp_[:, g0:512],
                                start=(jj == 0), stop=False)
                            drain_po(1)
                        if j == 3 and pending_den is not None:
                            den_chain(*pending_den)
                            pending_den = None
                    for (jj, pp_) in pj:
                        g0 = max(0, jj * 128 - half * 512) if TRIM else 0
                        nc.tensor.matmul(
                            pv[:, g0:512], v_t[:, jj, :], pp_[:, g0:512],
                            start=(jj == 0), stop=(jj == jmax - 1))
                        drain_po(1)
                    pending_den = (h, acc, pv)
                if n == NT - 1:
                    den_chain(*pending_den)
                    drain_po(32, force=True)
                else:
                    drain_po(3, force=True)
                    den_chain(*pending_den)
                    drain_po(32, force=True)
                prev = ((at_h, at_l), n)

            # epilogue: o_proj for the last tile
            po_queue = list(range(32))
            po_state = prev
            while po_queue:
                emit_oproj_group(po_state, po_queue.pop(0), alt=True)

    nc.compile()
    return nc


def _get_program():
    if "nc" not in _PROG:
        _PROG["nc"] = _build_program()
    return _PROG["nc"]


def _host_inputs(positions, hidden_states, w_qkv, w_o):
    positions = np.asarray(positions)
    hidden_states = np.asarray(hidden_states, dtype=np.float32)
    w_qkv = np.asarray(w_qkv, dtype=np.float32)
    w_o = np.asarray(w_o, dtype=np.float32)

    def f8(x):
        return x.astype(ml_dtypes.float8_e4m3)

    def f8v(x):
        return f8(x).astype(np.float32)

    hT = np.ascontiguousarray(hidden_states.reshape(TOK, H).T)
    hTh = f8(hT)                                    # h1
    hTl = f8(f8v(hT) + GQ * (hT - f8v(hT)))         # t_h = Q8(h1 + G*h0)

    pos0 = positions[0].astype(np.float32)
    inv = 1.0 / (THETA ** (np.arange(64, dtype=np.float32) / 64.0))
    ang = inv[:, None] * pos0[None, :]            # [64, S]
    c = np.cos(ang).astype(np.float32) * (QC / WQ_SCALE)
    s = np.sin(ang).astype(np.float32) * (QC / WQ_SCALE)
    cs = np.concatenate([c, c], axis=0).astype(ml_dtypes.bfloat16)
    sn = np.concatenate([-s, s], axis=0).astype(ml_dtypes.bfloat16)

    p = np.arange(128)[:, None]
    f = np.arange(512)[None, :]
    mk = np.zeros((4, 128, 512), dtype=ml_dtypes.bfloat16)
    for mi in range(4):
        mk[mi] = (p + 128 * mi <= f).astype(ml_dtypes.bfloat16)
    ones = np.ones((128, 128), dtype=np.float32)

    in_maps = []
    for c_ in range(N_CORES):
        q0 = c_ * NHL * D
        kc = NH * D + c_ * D
        vc = NH * D + NKV * D + c_ * D
        wq = np.ascontiguousarray(np.concatenate(
            [w_qkv[:, q0:q0 + NHL * D],
             w_qkv[:, kc:kc + D],
             w_qkv[:, vc:vc + D]], axis=1)) * WQ_SCALE
        wqh = f8(wq)                                # w1
        # t_w'' = Q8((w1 + G*w0)/(G-1)): folds the 1/(G-1) combine weight
        wql = f8((f8v(wq) + GQ * (wq - f8v(wq))) / (GQ - 1.0))
        wo = np.ascontiguousarray(
            w_o[c_ * WO_K:(c_ + 1) * WO_K, :]) * WQ_SCALE
        woh = f8(wo)
        wol = f8((f8v(wo) + GO * (wo - f8v(wo))) / (GO - 1.0))
        in_maps.append({"hTh": hTh, "hTl": hTl, "wqh": wqh,
                        "wql": wql, "woh": woh, "wol": wol, "cs": cs, "sn": sn,
                        "mk": mk, "on": ones})
    return in_maps


def run(positions, hidden_states, w_qkv, w_o, trace=False):
    from concourse import bass_utils
    nc = _get_program()
    in_maps = _host_inputs(positions, hidden_states, w_qkv, w_o)
    res = bass_utils.run_bass_kernel_spmd(
        nc, in_maps, core_ids=list(range(N_CORES)), trace=trace)
    acc = np.zeros((H, TOK), dtype=np.float32)
    for c in range(N_CORES):
        acc += np.asarray(res.results[c]["outT"], dtype=np.float32)
    acc *= OC                     # (1-1/GO) Karatsuba weight for o_proj
    out = np.ascontiguousarray(acc.T).reshape(B, S, H)
    return out, res


def kernel(positions, hidden_states, w_qkv, w_o):
    out, _ = run(positions, hidden_states, w_qkv, w_o, trace=False)
    return out



# revision 57
# speedup vs baseline: 1.2589x; 1.0266x over previous
"""Llama GQA attention (B=4,S=1024,H=4096,NH=32,NKV=8,D=128) on 8 TRN2 cores.

Strategy: tensor-parallel over heads (4 q heads + 1 kv head per core), host
all-reduce of o_proj partials.  v3: Karatsuba-gamma 2-slot fp8 matmuls.

Projection matmuls (qkv and o_proj) use a 2-term fp8e4m3 decomposition
instead of the classic 3-term hi/lo cross products:
    x*w ~ (1-1/G) * [Q(x)@Q(w) + t_x@t_w'']
    t_x  = Q8(x1 + G*(x - x1)),  t_w'' = Q8((w1 + G*(w - w1))/(G-1))
The 1/(G-1) combine weight folds into the host-side quantization of the
t-weights (so both terms accumulate in one PSUM chain with unit weights),
and the global (1-1/G) folds into the RoPE tables / v-evict scale (qkv)
and the host reduction (o_proj).  At G=6 this gives ~1.2% RMS per matmul
(vs 3.6% for a naive 2-term drop), rel err ~1.3e-2 < 2e-2, while cutting
PE slots 48->32 (qkv) and 6->4 (o_proj) per output tile: ~137us less PE.

Per 512-token tile n (batch b = n//2, half = n%2):
  1. qkv^T 2-term matmuls (fp8 DoubleRow, m-outer k-inner, 2 PSUM banks)
  2. RoPE epilogues per m as banks complete (4-op rotate-half on DVE);
     v evicted via DVE tensor_scalar + xbar DMA-transpose
  3. causal attention per head with column-trimmed score AND PV matmuls
     (bf16), exp on Act, diagonal-block mask on DVE, softmax denominator
     on Pool; den_chain also emits at_h=Q8(attn) and the Karatsuba t-term
     t_a = at_h + G*(attn - at_h) (residual TT on Pool, STT on DVE --
     gpsimd may not touch PSUM and has no TensorScalarPtr opcode on HW)
  4. o_proj matmuls of tile n-1 interleaved between attention matmuls;
     final epilogue rotates its PSUMs across all four (then-idle) pools
     so the 2-bank po ring never gates the drain.
"""

import numpy as np
import ml_dtypes

B, S, H = 4, 1024, 4096
NH, NKV, D = 32, 8, 128
THETA = 10000.0
N_CORES = 8
NHL = NH // N_CORES            # 4 local q heads
TOK = B * S                    # 4096 tokens
NT = TOK // 512                # 8 token tiles
KT = H // 128                  # 32 contraction tiles for qkv
QKV_COLS = (NHL + 2) * D       # 768 local qkv columns
WO_K = NHL * D                 # 512 local o_proj contraction
SCALE = 1.0 / float(np.sqrt(D))
WQ_SCALE = 64.0
TRIM = True
POOL_ACC = True
# Karatsuba-gamma 2-slot fp8 decomposition: x*w ~ (1-1/G) * [x1@w1 +
# (x1+G*x0)@Q((w1+G*w0)/(G-1))].  The 1/(G-1) folds into the host-side
# quantization of the t-weights; the global (1-1/G) folds into the RoPE
# tables / v-evict scale (qkv) and the host reduction (o_proj).
GQ = 6.0                       # gamma for qkv projection
GO = 6.0                       # gamma for o_proj
QC = 1.0 - 1.0 / GQ
OC = 1.0 - 1.0 / GO


_PROG = {}
MM_LABELS = []


def _lbl(tag):
    MM_LABELS.append(tag)



def _build_program():
    import concourse.mybir as mybir
    import concourse.bass_isa as bass_isa
    import concourse.tile as tile
    from concourse import bacc

    F32 = mybir.dt.float32
    F32R = mybir.dt.float32r
    BF16 = mybir.dt.bfloat16
    MUL = mybir.AluOpType.mult
    ADD = mybir.AluOpType.add
    EXP = mybir.ActivationFunctionType.Exp
    DR = mybir.MatmulPerfMode.DoubleRow

    nc = bacc.Bacc("TRN2", target_bir_lowering=False, debug=False,
                   num_devices=N_CORES)

    F8 = mybir.dt.float8e4
    hTh_d = nc.dram_tensor("hTh", (H, TOK), F8, kind="ExternalInput")
    hTl_d = nc.dram_tensor("hTl", (H, TOK), F8, kind="ExternalInput")
    wqh_d = nc.dram_tensor("wqh", (H, QKV_COLS), F8, kind="ExternalInput")
    wql_d = nc.dram_tensor("wql", (H, QKV_COLS), F8, kind="ExternalInput")
    woh_d = nc.dram_tensor("woh", (WO_K, H), F8, kind="ExternalInput")
    wol_d = nc.dram_tensor("wol", (WO_K, H), F8, kind="ExternalInput")
    cs_d = nc.dram_tensor("cs", (128, S), BF16, kind="ExternalInput")  # [cos;cos]
    sn_d = nc.dram_tensor("sn", (128, S), BF16, kind="ExternalInput")  # [-sin;sin]
    mk_d = nc.dram_tensor("mk", (4, 128, 512), BF16, kind="ExternalInput")
    on_d = nc.dram_tensor("on", (128, 128), F32R, kind="ExternalInput")
    outT_d = nc.dram_tensor("outT", (H, TOK), BF16, kind="ExternalOutput")

    hTh_r = hTh_d.rearrange("(kp two ki) t -> ki kp two t", ki=128, two=2)
    hTl_r = hTl_d.rearrange("(kp two ki) t -> ki kp two t", ki=128, two=2)
    wqh_r = wqh_d.rearrange("(kp two ki) c -> ki kp two c", ki=128, two=2)
    wql_r = wql_d.rearrange("(kp two ki) c -> ki kp two c", ki=128, two=2)
    woh_r = woh_d.rearrange("(kp two ki) m -> ki kp two m", ki=128, two=2)
    wol_r = wol_d.rearrange("(kp two ki) m -> ki kp two m", ki=128, two=2)
    outT_r = outT_d.rearrange("(mo ki) t -> ki mo t", ki=128)

    with nc.allow_low_precision(reason="bf16 compute within 2e-2 tolerance"), \
         tile.TileContext(nc) as tc:
        with tc.tile_pool(name="persist", bufs=1) as pp, \
             tc.tile_pool(name="io2", bufs=2) as io2, \
             tc.tile_pool(name="sb2", bufs=2) as sb2, \
             tc.tile_pool(name="sb3", bufs=2) as sb3, \
             tc.tile_pool(name="sb1", bufs=1) as sb1, \
             tc.tile_pool(name="ost", bufs=5) as ost, \
             tc.tile_pool(name="probs", bufs=5) as pprob, \
             tc.tile_pool(name="ps_qkv", bufs=2, space="PSUM") as ps_qkv, \
             tc.tile_pool(name="ps_s", bufs=2, space="PSUM") as ps_sp, \
             tc.tile_pool(name="ps_pv", bufs=2, space="PSUM") as ps_pv, \
             tc.tile_pool(name="ps_po", bufs=2, space="PSUM") as ps_po:

            # ---- weights + tables; wq k-groups interleaved with tile-0 hT
            # chunks so the first matmuls start as soon as (wq_g0, hT_c0)
            # land; small tables next (needed ~40us in); wo (needed only
            # from tile 1) last.
            hTh0 = io2.tile([128, 16, 2, 512], F8, tag="hTh")
            hTl0 = io2.tile([128, 16, 2, 512], F8, tag="hTl")
            wq_g = []
            for g in range(4):
                th = pp.tile([128, 4, 2, QKV_COLS], F8, name=f"wqh{g}")
                tl = pp.tile([128, 4, 2, QKV_COLS], F8, name=f"wql{g}")
                gp = slice(g * 4, (g + 1) * 4)
                if g == 0:
                    # v/k weight columns + first hT chunks first, in the
                    # order the 2-term matmul loop consumes them (w1 and
                    # t_w both needed from kp=0).  The t-stream loads go out
                    # on the Act HWDGE queue (idle at startup) so descriptor
                    # generation runs in parallel with the SP queue.
                    nc.sync.dma_start(th[:, :, :, 512:768],
                                      wqh_r[:, gp, :, 512:768])
                    nc.sync.dma_start(tl[:, :, :, 512:768],
                                      wql_r[:, gp, :, 512:768])
                    nc.sync.dma_start(hTh0[:, 0:2, :, :],
                                      hTh_r[:, 0:2, :, 0:512])
                    nc.sync.dma_start(hTl0[:, 0:2, :, :],
                                      hTl_r[:, 0:2, :, 0:512])
                    nc.sync.dma_start(hTh0[:, 2:4, :, :],
                                      hTh_r[:, 2:4, :, 0:512])
                    nc.sync.dma_start(hTl0[:, 2:4, :, :],
                                      hTl_r[:, 2:4, :, 0:512])
                    nc.sync.dma_start(th[:, :, :, 0:512],
                                      wqh_r[:, gp, :, 0:512])
                    nc.sync.dma_start(tl[:, :, :, 0:512],
                                      wql_r[:, gp, :, 0:512])
                else:
                    g2a = slice(g * 4, g * 4 + 2)
                    g2b = slice(g * 4 + 2, g * 4 + 4)
                    nc.sync.dma_start(th[:], wqh_r[:, gp, :, :])
                    nc.sync.dma_start(hTh0[:, g2a, :, :],
                                      hTh_r[:, g2a, :, 0:512])
                    nc.sync.dma_start(tl[:], wql_r[:, gp, :, :])
                    nc.sync.dma_start(hTl0[:, g2a, :, :],
                                      hTl_r[:, g2a, :, 0:512])
                    nc.sync.dma_start(hTh0[:, g2b, :, :],
                                      hTh_r[:, g2b, :, 0:512])
                    nc.sync.dma_start(hTl0[:, g2b, :, :],
                                      hTl_r[:, g2b, :, 0:512])
                wq_g.append((th, tl))
            cs_t = pp.tile([128, S], BF16)
            nc.sync.dma_start(cs_t[:], cs_d[:])
            sn_t = pp.tile([128, S], BF16)
            nc.sync.dma_start(sn_t[:], sn_d[:])
            mk_t = pp.tile([128, 4, 512], BF16)
            nc.sync.dma_start(mk_t[:], mk_d.rearrange("m p f -> p m f"))
            on_t = pp.tile([128, 128], F32R)
            nc.sync.dma_start(on_t[:], on_d[:])
            woh_sb = pp.tile([128, 2, 2, H], F8)
            wol_sb = pp.tile([128, 2, 2, H], F8)

            probs_bufs = [pprob.tile([128, 512], BF16, tag="probs",
                                     name=f"pz{i}") for i in range(5)]
            for t_ in probs_bufs:
                nc.vector.memset(t_[:], 0.0)

            # ---- per-tile state (python-side handles) ----
            kT_t = None      # [128, 1024] bf16, k^T for current batch
            v_t = None       # [128, 8, 128] bf16, v natural for current batch
            prev = None      # (attnT tile, token-tile index) pending o_proj
            carry = None     # tile-1 qkv units pre-built at tile 0
            hT_tiles = {0: (hTh0, hTl0)}

            def emit_oproj_group(state, g, alt=False):
                """One o_proj m-tile: 4 matmuls + evict + maybe DMA."""
                (ah_p, al_p), n_p = state
                mi = g % 2
                if mi == 0:
                    _ostage[0] = ost.tile([128, 2, 512], BF16, tag="ostage",
                                          name="ostage")
                stage = _ostage[0]
                if alt:
                    # final epilogue: every other psum pool is idle now --
                    # rotate across all 8 banks to hide evict latency
                    pp_, tg = ((ps_po, "po"), (ps_qkv, "qkv"),
                               (ps_sp, "s"), (ps_pv, "pv"))[g % 4]
                    po = pp_.tile([128, 512], F32, tag=tg, name="po")
                else:
                    po = ps_po.tile([128, 512], F32, tag="po")
                ms = slice(g * 128, (g + 1) * 128)
                st = True
                for p_ in range(2):
                    hp = slice(2 * p_, 2 * p_ + 2)
                    for wt, at in ((woh_sb, ah_p), (wol_sb, al_p)):
                        _lbl(f"oproj t{n_p} g{g}")
                        nc.tensor.matmul(
                            po[:], wt[:, p_, :, ms], at[:, hp, :],
                            start=st, stop=(p_ == 1 and wt is wol_sb),
                            perf_mode=DR)
                        st = False
                if alt and g % 2:
                    nc.vector.tensor_scalar(stage[:, mi, :], po[:],
                                            1.0 / WQ_SCALE, None, op0=MUL)
                else:
                    nc.scalar.mul(stage[:, mi, :], po[:], 1.0 / WQ_SCALE)
                if mi == 1:
                    mg = g // 2
                    nc.sync.dma_start(
                        outT_r[:, mg * 2:(mg + 1) * 2,
                               n_p * 512:(n_p + 1) * 512], stage[:])

            _ostage = [None]

            for n in range(NT):
                b, half = n // 2, n % 2
                csl = cs_t[:, half * 512:(half + 1) * 512]
                snl = sn_t[:, half * 512:(half + 1) * 512]

                # queue of pending o_proj groups for tile n-1
                po_queue = list(range(32)) if prev is not None else []
                po_state = prev
                po_cap = [0]

                def drain_po(k=1, force=False):
                    for _ in range(k):
                        if po_queue and (force or 32 - len(po_queue) < po_cap[0]):
                            emit_oproj_group(po_state, po_queue.pop(0))
                        elif fill_q:
                            fill_q.pop(0)()

                # ---- prefetch next tile's hT (double-buffered) ----
                if n + 1 < NT:
                    nxh = io2.tile([128, 16, 2, 512], F8, tag="hTh",
                                   name="hTnh")
                    nxl = io2.tile([128, 16, 2, 512], F8, tag="hTl",
                                   name="hTnl")
                    tsl = slice((n + 1) * 512, (n + 2) * 512)
                    for g in range(4):
                        gp = slice(g * 4, (g + 1) * 4)
                        nc.sync.dma_start(nxh[:, gp, :, :],
                                          hTh_r[:, gp, :, tsl])
                        nc.sync.dma_start(nxl[:, gp, :, :],
                                          hTl_r[:, gp, :, tsl])
                    hT_tiles[n + 1] = (nxh, nxl)
                hTh_t, hTl_t = hT_tiles.pop(n)

                # ---- qkv + RoPE ----
                fill_q = []
                if n == 1 and carry:
                    qT_t, kT_t, v_t = carry["qkv_out"]
                else:
                    qT_t = sb2.tile([128, NHL, 512], BF16, tag="qT")
                    if half == 0:
                        kT_t = sb2.tile([128, S], BF16, tag="kT")
                        v_t = sb2.tile([128, 8, 128], BF16, tag="v")

                def qkv_epilogue(m, ps):
                    if m < NHL or m == 4:
                        # RoPE: out = ps*[cos;cos] + rot(ps)*[-sin;sin]
                        if m < NHL:
                            out = qT_t[:, m, :]
                        else:
                            out = kT_t[:, half * 512:(half + 1) * 512]
                        tco = sb1.tile([128, 512], BF16, tag="tco")
                        tsi = sb1.tile([128, 512], BF16, tag="tsi")
                        nc.vector.tensor_tensor(tco[:], ps[:], csl, op=MUL)
                        nc.vector.tensor_tensor(tsi[0:64, :], ps[64:128, :],
                                                snl[0:64, :], op=MUL)
                        nc.vector.tensor_tensor(tsi[64:128, :], ps[0:64, :],
                                                snl[64:128, :], op=MUL)
                        nc.vector.tensor_tensor(out, tco[:], tsi[:], op=ADD)
                    else:
                        # v: evict bf16, then xbar DMA-transpose to [tok, d]
                        vT_tmp = sb1.tile([128, 512], BF16, tag="vT")
                        nc.vector.tensor_scalar(vT_tmp[:], ps[:],
                                                QC / WQ_SCALE, None, op0=MUL)
                        for c4 in range(4):
                            nc.sync.dma_start(
                                v_t[:, half * 4 + c4, :],
                                vT_tmp[:, c4 * 128:(c4 + 1) * 128],
                                transpose=True)

                M_ORDER = (5, 4, 0, 1, 2, 3)  # v,k first: their consumers
                # sit at the head of the attention phase
                if n == 0:
                    # startup: g-outer so matmuls start as DMA chunks land;
                    # 6 concurrent banks borrowed from the idle s/pv pools
                    banks = {5: ps_qkv.tile([128, 512], F32, tag="qkv", name="b5"),
                             4: ps_qkv.tile([128, 512], F32, tag="qkv", name="b4"),
                             0: ps_sp.tile([128, 512], F32, tag="s", name="b0"),
                             1: ps_sp.tile([128, 512], F32, tag="s", name="b1"),
                             2: ps_pv.tile([128, 512], F32, tag="pv", name="b2"),
                             3: ps_pv.tile([128, 512], F32, tag="pv", name="b3")}
                    for g in range(4):
                        th, tl = wq_g[g]
                        for m in M_ORDER:
                            ms = slice(m * 128, (m + 1) * 128)
                            for kk in range(4):
                                kp = g * 4 + kk
                                st = kp == 0
                                for wt, ht in ((th, hTh_t), (tl, hTl_t)):
                                    _lbl(f"qkv0 m{m}")
                                    nc.tensor.matmul(
                                        banks[m][:], wt[:, kk, :, ms],
                                        ht[:, kp, :, :], start=st,
                                        stop=(kp == 15 and wt is tl),
                                        perf_mode=DR)
                                    st = False
                    for m in M_ORDER:
                        qkv_epilogue(m, banks[m])
                elif n == 1 and carry:
                    rest = carry["units"]
                    while rest:
                        rest.pop(0)()
                    carry = None
                else:
                    for m in M_ORDER:
                        ms = slice(m * 128, (m + 1) * 128)
                        ps = ps_qkv.tile([128, 512], F32, tag="qkv")
                        for kp in range(16):
                            th, tl = wq_g[kp // 4]
                            kk = kp % 4
                            st = kp == 0
                            for wt, ht in ((th, hTh_t), (tl, hTl_t)):
                                _lbl(f"qkv n{n} m{m}")
                                nc.tensor.matmul(
                                    ps[:], wt[:, kk, :, ms],
                                    ht[:, kp, :, :], start=st,
                                    stop=(kp == 15 and wt is tl),
                                    perf_mode=DR)
                                st = False
                        qkv_epilogue(m, ps)

                if n == 0:
                    # pre-build tile-1 qkv units; drain into tile-0
                    # attention gaps (no o_proj exists yet)
                    n1h, n1l = hT_tiles[1]
                    qT1 = sb2.tile([128, NHL, 512], BF16, tag="qT",
                                   name="qT1")
                    cs1 = cs_t[:, 512:1024]
                    sn1 = sn_t[:, 512:1024]
                    st1 = {}

                    def mk_unit(m, kp):
                        def emit():
                            ms = slice(m * 128, (m + 1) * 128)
                            if kp == 0:
                                st1[m] = ps_qkv.tile([128, 512], F32,
                                                     tag="qkv", name="q1")
                            ps = st1[m]
                            th, tl = wq_g[kp // 4]
                            kk = kp % 4
                            st = kp == 0
                            for wt, ht in ((th, n1h), (tl, n1l)):
                                _lbl(f"qkv1c m{m}")
                                nc.tensor.matmul(
                                    ps[:], wt[:, kk, :, ms],
                                    ht[:, kp, :, :], start=st,
                                    stop=(kp == 15 and wt is tl),
                                    perf_mode=DR)
                                st = False
                            if kp == 15:
                                if m < NHL:
                                    out = qT1[:, m, :]
                                else:
                                    out = kT_t[:, 512:1024]
                                if m < NHL or m == 4:
                                    tco = sb1.tile([128, 512], BF16,
                                                   tag="tco", name="tc1")
                                    tsi = sb1.tile([128, 512], BF16,
                                                   tag="tsi", name="ts1")
                                    nc.vector.tensor_tensor(
                                        tco[:], ps[:], cs1, op=MUL)
                                    nc.vector.tensor_tensor(
                                        tsi[0:64, :], ps[64:128, :],
                                        sn1[0:64, :], op=MUL)
                                    nc.vector.tensor_tensor(
                                        tsi[64:128, :], ps[0:64, :],
                                        sn1[64:128, :], op=MUL)
                                    nc.vector.tensor_tensor(
                                        out, tco[:], tsi[:], op=ADD)
                                else:
                                    vT_tmp = sb1.tile([128, 512], BF16,
                                                      tag="vT", name="v1")
                                    nc.scalar.mul(vT_tmp[:], ps[:],
                                                  QC / WQ_SCALE)
                                    for c4 in range(4):
                                        nc.sync.dma_start(
                                            v_t[:, 4 + c4, :],
                                            vT_tmp[:, c4 * 128:
                                                   (c4 + 1) * 128],
                                            transpose=True)
                        return emit

                    units = [mk_unit(m, kp) for m in M_ORDER
                             for kp in range(16)]
                    fill_q = units
                    carry = {"units": units,
                             "qkv_out": (qT1, kT_t, v_t)}
                    # wo loads behind the tile-1 hT prefetch; first o_proj
                    # consumer is a full tile away
                    nc.sync.dma_start(woh_sb[:], woh_r[:])
                    nc.sync.dma_start(wol_sb[:], wol_r[:])

                # ---- attention (+ interleaved o_proj of tile n-1) ----
                jmax = 4 + half * 4
                at_h = sb2.tile([128, NHL, 512], F8, tag="at_h")
                at_l = sb2.tile([128, NHL, 512], F8, tag="at_l")

                def den_chain(h, acc, pv):
                    # softmax denominator (partition all-reduce broadcasts
                    # the column sum to every partition) / normalization
                    dbc = sb1.tile([128, 512], F32R, tag="dbc", name="dbc")
                    nc.gpsimd.partition_all_reduce(
                        dbc[:], acc[:], 128, reduce_op=bass_isa.ReduceOp.add)
                    drain_po(2)
                    rbc = sb1.tile([128, 512], F32R, tag="rbc", name="rbc")
                    nc.vector.reciprocal(rbc[:], dbc[:])
                    at_t = sb1.tile([128, 512], BF16, tag="at_t",
                                    name="at_t")
                    nc.vector.tensor_tensor(at_t[:], pv[:], rbc[:], op=MUL)
                    nc.vector.tensor_copy(at_h[:, h, :], at_t[:])
                    # t_a = at_h + GO*(at_t - at_h), fp8 (Karatsuba t-term).
                    # Off the DVE critical chain: both ops on idle gpsimd.
                    tml = sb1.tile([128, 512], BF16, tag="tml", name="tml")
                    nc.vector.tensor_tensor(tml[:], at_t[:], at_h[:, h, :],
                                            op=mybir.AluOpType.subtract)
                    drain_po(1)
                    # STT is not a legal Pool opcode on HW; keep it on DVE
                    nc.vector.scalar_tensor_tensor(
                        at_l[:, h, :], tml[:], GO, at_h[:, h, :],
                        op0=MUL, op1=ADD)
                    drain_po(2)

                pending_den = None
                for h in range(NHL):
                    po_cap[0] = 8 * (h + 1) if h < NHL - 1 else (26 if half else 23)
                    acc = sb2.tile([128, 512], F32R, tag="acc")
                    pv = ps_pv.tile([128, 512], F32, tag="pv")
                    pj = []  # pending probs for PV (lag 2)
                    for j in range(jmax):
                        mf0 = max(0, j * 128 - half * 512)
                        f0 = mf0 if TRIM else 0
                        w = 512 - f0
                        sps = ps_sp.tile([128, 512], F32, tag="s")
                        _lbl(f"score n{n} h{h} j{j}")
                        nc.tensor.matmul(
                            sps[:, 0:w], kT_t[:, j * 128:(j + 1) * 128],
                            qT_t[:, h, f0:512], start=True, stop=True)
                        probs = pprob.tile([128, 512], BF16, tag="probs")
                        nc.scalar.activation(probs[:, f0:512], sps[:, 0:w],
                                             EXP, scale=SCALE)
                        if j * 128 >= half * 512:
                            # diagonal block: full-width mask zeroes
                            # probs[:, :mf0] (stale) and the upper triangle
                            nc.vector.tensor_tensor(
                                probs[:], probs[:],
                                mk_t[:, j - half * 4, :], op=MUL)
                        acc_eng = (nc.gpsimd if POOL_ACC and h < NHL - 1
                                   else nc.vector)
                        if j == 0:
                            acc_eng.tensor_copy(acc[:], probs[:])
                        else:
                            acc_eng.tensor_tensor(
                                acc[:, mf0:512], acc[:, mf0:512],
                                probs[:, mf0:512], op=ADD)
                        pj.append((j, probs))
                        drain_po(1)
                        if len(pj) >= 4:
                            jj, pp_ = pj.pop(0)
                            g0 = max(0, jj * 128 - half * 512) if TRIM else 0
                            _lbl(f"pv n{n} h{h} j{jj}")
                            nc.tensor.matmul(
                                pv[:, g0:512], v_t[:, jj, :], pp_[:, g0:512],
                                start=(jj == 0), stop=False)
                            drain_po(1)
                        if j == 3 and pending_den is not None:
                            den_chain(*pending_den)
                            pending_den = None
                    for (jj, pp_) in pj:
                        g0 = max(0, jj * 128 - half * 512) if TRIM else 0
                        _lbl(f"pv n{n} h{h} j{jj}")
                        nc.tensor.matmul(
                            pv[:, g0:512], v_t[:, jj, :], pp_[:, g0:512],
                            start=(jj == 0), stop=(jj == jmax - 1))
                        drain_po(1)
                    pending_den = (h, acc, pv)
                den_chain(*pending_den)
                drain_po(32, force=True)
                prev = ((at_h, at_l), n)

            # epilogue: o_proj for the last tile
            po_queue = list(range(32))
            po_state = prev
            while po_queue:
                emit_oproj_group(po_state, po_queue.pop(0), alt=True)

    nc.compile()
    return nc


def _get_program():
    if "nc" not in _PROG:
        _PROG["nc"] = _build_program()
    return _PROG["nc"]


def _host_inputs(positions, hidden_states, w_qkv, w_o):
    positions = np.asarray(positions)
    hidden_states = np.asarray(hidden_states, dtype=np.float32)
    w_qkv = np.asarray(w_qkv, dtype=np.float32)
    w_o = np.asarray(w_o, dtype=np.float32)

    def f8(x):
        return x.astype(ml_dtypes.float8_e4m3)

    def f8v(x):
        return f8(x).astype(np.float32)

    hT = np.ascontiguousarray(hidden_states.reshape(TOK, H).T)
    hTh = f8(hT)                                    # h1
    hTl = f8(f8v(hT) + GQ * (hT - f8v(hT)))         # t_h = Q8(h1 + G*h0)

    pos0 = positions[0].astype(np.float32)
    inv = 1.0 / (THETA ** (np.arange(64, dtype=np.float32) / 64.0))
    ang = inv[:, None] * pos0[None, :]            # [64, S]
    c = np.cos(ang).astype(np.float32) * (QC / WQ_SCALE)
    s = np.sin(ang).astype(np.float32) * (QC / WQ_SCALE)
    cs = np.concatenate([c, c], axis=0).astype(ml_dtypes.bfloat16)
    sn = np.concatenate([-s, s], axis=0).astype(ml_dtypes.bfloat16)

    p = np.arange(128)[:, None]
    f = np.arange(512)[None, :]
    mk = np.zeros((4, 128, 512), dtype=ml_dtypes.bfloat16)
    for mi in range(4):
        mk[mi] = (p + 128 * mi <= f).astype(ml_dtypes.bfloat16)
    ones = np.ones((128, 128), dtype=np.float32)

    in_maps = []
    for c_ in range(N_CORES):
        q0 = c_ * NHL * D
        kc = NH * D + c_ * D
        vc = NH * D + NKV * D + c_ * D
        wq = np.ascontiguousarray(np.concatenate(
            [w_qkv[:, q0:q0 + NHL * D],
             w_qkv[:, kc:kc + D],
             w_qkv[:, vc:vc + D]], axis=1)) * WQ_SCALE
        wqh = f8(wq)                                # w1
        # t_w'' = Q8((w1 + G*w0)/(G-1)): folds the 1/(G-1) combine weight
        wql = f8((f8v(wq) + GQ * (wq - f8v(wq))) / (GQ - 1.0))
        wo = np.ascontiguousarray(
            w_o[c_ * WO_K:(c_ + 1) * WO_K, :]) * WQ_SCALE
        woh = f8(wo)
        wol = f8((f8v(wo) + GO * (wo - f8v(wo))) / (GO - 1.0))
        in_maps.append({"hTh": hTh, "hTl": hTl, "wqh": wqh,
                        "wql": wql, "woh": woh, "wol": wol, "cs": cs, "sn": sn,
                        "mk": mk, "on": ones})
    return in_maps


def run(positions, hidden_states, w_qkv, w_o, trace=False):
    from concourse import bass_utils
    nc = _get_program()
    in_maps = _host_inputs(positions, hidden_states, w_qkv, w_o)
    res = bass_utils.run_bass_kernel_spmd(
        nc, in_maps, core_ids=list(range(N_CORES)), trace=trace)
    acc = np.zeros((H, TOK), dtype=np.float32)
    for c in range(N_CORES):
        acc += np.asarray(res.results[c]["outT"], dtype=np.float32)
    acc *= OC                     # (1-1/GO) Karatsuba weight for o_proj
    out = np.ascontiguousarray(acc.T).reshape(B, S, H)
    return out, res


def kernel(positions, hidden_states, w_qkv, w_o):
    out, _ = run(positions, hidden_states, w_qkv, w_o, trace=False)
    return out

